# revision 6
# baseline (speedup 1.0000x reference)
"""Trainium2 Bass kernel for the ChimeraSurrogateNCA problem.

Masked 3x3 conv NCA, 5 steps, B=4 C=256 H=W=128, softsign residual.

Sharding: 8 cores = 4 horizontal bands (32 rows) x 2 batch pairs.
Each core keeps its 2 batches' band + steps-row halo resident in SBUF
across all steps (redundant halo compute, zero inter-core comms).
Wider bands cut the halo-redundancy overhead from 25% to 19% vs the
16-row x 4-batch split.

Layout: x stored [cin -> 2x128 partition blocks, flat (row*W + col)]
in fp16 with NO pad columns (W=128 row pitch). A 3x3 tap's dx shift is
a +-1 element offset in the flat buffer; positions that wrap into a
neighboring row are zeroed by the causal-mask tiles (host pre-shifted
per tap, 0 outside the image). Every DVE/ACT op is fully contiguous.
Masks are stored fp8e4m3 (0/1 exact) to fit the 42-row slab in SBUF.

Engine split per 4-row group (tuned against the NTFF trace):
  - mask multiplies (8 taps x 2 cin blocks) on DVE only: GPSIMD shares
    an SBUF port with the DVE, so offloading tensor_tensor work there
    slows 2-port DVE ops ~3.2x (measured) and is net-negative
  - matmuls (fp16, fp32 PSUM accum) on PE; fp8 fails accuracy (2.5-5%
    vs the 2e-2 budget, simulated)
  - softsign residual: |d| on ACT, then 1/(1+|d|) = exp(-ln(|d|+1))
    on ACT (Ln's pre-bias fuses the +1; Ln/Exp/Abs live in one ACT
    table set so there is exactly one ACT_TABLE_LOAD), d*r and the
    in-place slab add on DVE (both contiguous).
"""

import numpy as np

import concourse.bass as bass
import concourse.mybir as mybir
from concourse.tile import TileContext

F16 = mybir.dt.float16
F32 = mybir.dt.float32
F8 = mybir.dt.float8e4

N_CORES = 8
B, C, H, W = 4, 256, 128, 128
P = 128          # partitions / channel block size
CB = C // P      # channel blocks (2)
OWN = 32         # band rows owned per core
N_BANDS = H // OWN   # 4
B_LOC = 2        # batches per core; core = band * 2 + (batch pair)
G = 4            # guard elements on each end of a slab/mask row buffer

# taps excluding the always-unmasked center (k=4)
TAPS = [0, 1, 2, 3, 5, 6, 7, 8]


def _build_program(S, hoist=True):
    SR = OWN + 2 * S          # slab rows (42 for S=5)
    LS = G + SR * W + G       # slab flat length per partition
    nc = bass.Bass()
    xin = nc.declare_dram_parameter("xin", [B_LOC, CB, P, LS], F16, isOutput=False)
    mk = nc.declare_dram_parameter("mk", [P, 8 * LS], F8, isOutput=False)
    wt = nc.declare_dram_parameter("wt", [CB, P, 9 * CB * P], F16, isOutput=False)
    out = nc.declare_dram_parameter("out", [B_LOC, CB, P, OWN * W], F16, isOutput=True)

    with TileContext(nc) as tc:
        with (
            tc.tile_pool(name="xp", bufs=1) as xpool,
            tc.tile_pool(name="mp", bufs=1) as mpool,
            tc.tile_pool(name="wp", bufs=1) as wpool,
            tc.tile_pool(name="ap", bufs=2) as apool,
            tc.tile_pool(name="tp", bufs=3) as tpool,
            tc.tile_pool(name="pp", bufs=2, space="PSUM") as ppool,
        ):
            slab = {}
            for b in range(B_LOC):
                for cb in range(CB):
                    t = xpool.tile([P, LS], F16, tag=f"slab{b}{cb}")
                    nc.sync.dma_start(out=t[:], in_=xin[b, cb])
                    slab[b, cb] = t
            mk_sb = mpool.tile([P, 8 * LS], F8, tag="mk")
            nc.sync.dma_start(out=mk_sb[:], in_=mk[:])
            w_sb = []
            for cb in range(CB):
                t = wpool.tile([P, 9 * CB * P], F16, tag=f"w{cb}")
                nc.sync.dma_start(out=t[:], in_=wt[cb])
                w_sb.append(t)

            def w_view(k, cb, ob):
                return w_sb[cb][:, (k * CB + ob) * P:(k * CB + ob + 1) * P]

            def emit_abuild(b, r0, R):
                # a[kk,cb][i] = slab[(r0+dy-1)*W - 2 + i] * mask'[same pos]
                tiles = {}
                for kk, k in enumerate(TAPS):
                    dy = k // 3
                    o = G - 2 + (r0 + dy - 1) * W
                    n = R * W + 4
                    for cb in range(CB):
                        at = apool.tile([P, R * W + 2 * G], F16, tag=f"a{k}{cb}")
                        nc.vector.tensor_tensor(
                            out=at[:, 0:n],
                            in0=slab[b, cb][:, o:o + n],
                            in1=mk_sb[:, kk * LS + o:kk * LS + o + n],
                            op=mybir.AluOpType.mult,
                        )
                        tiles[k, cb] = at
                return tiles

            def emit_center(b, r0, R, psums):
                for ob in range(CB):
                    for cb in range(CB):
                        rhs = slab[b, cb][:, G + r0 * W:G + r0 * W + R * W]
                        nc.tensor.matmul(
                            psums[ob][:], w_view(4, cb, ob), rhs,
                            start=(cb == 0), stop=False,
                        )

            def emit_rest(b, r0, R, tiles, psums):
                for ob in range(CB):
                    n = 0
                    for k in TAPS:
                        dx = k % 3
                        for cb in range(CB):
                            n += 1
                            rhs = tiles[k, cb][:, 1 + dx:1 + dx + R * W]
                            nc.tensor.matmul(
                                psums[ob][:], w_view(k, cb, ob), rhs,
                                start=False, stop=(n == 2 * len(TAPS)),
                            )

            def emit_resid(b, r0, R, psums):
                # x += d/(1+|d|) with 1/(1+|d|) = exp(-ln(|d|+1)), all on ACT
                for ob in range(CB):
                    ps = psums[ob]
                    tabs = tpool.tile([P, R * W], F32, tag="tabs")
                    nc.scalar.activation(
                        out=tabs[:], in_=ps[:],
                        func=mybir.ActivationFunctionType.Abs,
                    )
                    v = tpool.tile([P, R * W], F32, tag="v")
                    nc.scalar.activation(
                        out=v[:], in_=tabs[:],
                        func=mybir.ActivationFunctionType.Ln, bias=1.0,
                    )
                    rt = tpool.tile([P, R * W], F32, tag="rt")
                    nc.scalar.activation(
                        out=rt[:], in_=v[:],
                        func=mybir.ActivationFunctionType.Exp, scale=-1.0,
                    )
                    st = tpool.tile([P, R * W], F16, tag="st")
                    nc.vector.tensor_tensor(
                        out=st[:], in0=ps[:], in1=rt[:], op=mybir.AluOpType.mult
                    )
                    sv = slab[b, ob][:, G + r0 * W:G + r0 * W + R * W]
                    nc.vector.tensor_tensor(
                        out=sv, in0=sv, in1=st[:], op=mybir.AluOpType.add
                    )

            for t in range(1, S + 1):
                lo, hi = t, SR - t
                for b in range(B_LOC):
                    groups = []
                    r = lo
                    while r < hi:
                        Rg = min(4, hi - r)
                        groups.append((r, Rg))
                        r += Rg
                    pending = None
                    for (r0, Rg) in groups:
                        tiles = emit_abuild(b, r0, Rg)
                        psums = [
                            ppool.tile([P, Rg * W], F32, tag=f"ps{ob}", name=f"ps{ob}")
                            for ob in range(CB)
                        ]
                        emit_center(b, r0, Rg, psums)
                        if pending is not None:
                            emit_resid(b, *pending)
                        emit_rest(b, r0, Rg, tiles, psums)
                        pending = (r0, Rg, psums)
                    emit_resid(b, *pending)

            for b in range(B_LOC):
                for cb in range(CB):
                    nc.sync.dma_start(
                        out=out[b, cb],
                        in_=slab[b, cb][:, G + S * W:G + (S + OWN) * W],
                    )
    if hoist:
        _hoist_extra_waits(nc)
    return nc


# Engine compute instructions have a single hardware sync-wait slot on
# trn2 (walrus: "Too many sync wait commands"); Tile may attach 2-3.
# Hoist the extras onto standalone EventSemaphore waits on the same
# engine queue immediately before the instruction.
_NO_HOIST = {
    "InstEventSemaphore", "InstCall",
    "InstUnconditionalBranch", "InstRegisterMove",
}


def _hoist_extra_waits(nc, max_waits=1):
    fn = nc.m.functions[0]
    n = 0
    for blk in fn.blocks:
        newlist = []
        for inst in blk.instructions:
            if (
                type(inst).__name__ == "InstISA"
                and getattr(inst, "op_name", "") == "EVENT_SEMAPHORE_RANGE_CLEAR"
            ):
                # kernel-tail lazy-sem reset; this walrus can't encode
                # opcode 176 ("ISA wrong length"). Only needed for NEFF
                # re-execution, which the runtime handles via fresh loads.
                continue
            si = inst.sync_info
            if (
                si is not None
                and si.on_wait
                and len(si.on_wait) > max_waits
                and type(inst).__name__ not in _NO_HOIST
            ):
                waits = list(si.on_wait)
                extra, keep = waits[:-max_waits], waits[-max_waits:]
                for j, wsub in enumerate(extra):
                    carrier = mybir.InstEventSemaphore(
                        name=f"hwait-{inst.name}-{j}", ins=[], outs=[]
                    )
                    carrier.engine = inst.engine
                    carrier.sync_info = type(si)(on_wait=[wsub], on_update=[])
                    newlist.append(carrier)
                    n += 1
                inst.sync_info = type(si)(
                    on_wait=keep, on_update=list(si.on_update or [])
                )
            newlist.append(inst)
        try:
            blk.instructions = newlist
        except Exception:
            blk.instructions[:] = newlist
    return n


def _pack_weights(Wt):
    # wt[cb][p, k*2*P + ob*P + co] = Wt[ob*P + co, cb*P + p, k]
    Wr = np.ascontiguousarray(Wt.reshape(C, C, 9))
    wta = Wr.reshape(CB, P, CB, P, 9)            # [ob, co, cb, p, k]
    wta = wta.transpose(2, 3, 4, 0, 1)           # [cb, p, k, ob, co]
    return np.ascontiguousarray(wta.reshape(CB, P, 9 * CB * P)).astype(np.float16)


def _pack_core_inputs(core, S, retina, mask, wt_host):
    import ml_dtypes

    SR = OWN + 2 * S
    LS = G + SR * W + G
    band, bp = core // 2, core % 2
    bsel = slice(bp * B_LOC, (bp + 1) * B_LOC)
    ir0 = band * OWN - S  # image row of slab row 0
    xin_host = np.zeros((B_LOC, CB, P, LS), np.float16)
    rlo = max(0, -ir0)
    rhi = min(SR, H - ir0)
    if rhi > rlo:
        xin_host[:, :, :, G + rlo * W:G + rhi * W] = (
            retina.reshape(B, CB, P, H, W)[bsel, :, :, ir0 + rlo:ir0 + rhi, :]
            .astype(np.float16).reshape(B_LOC, CB, P, (rhi - rlo) * W)
        )
    mk_host = np.zeros((8, LS), np.float32)
    for kk, k in enumerate(TAPS):
        dy, dx = k // 3, k % 3
        # mask multiplying slab input element (q, c) for tap k is
        # mask[k][ir0 + q - (dy-1), c - (dx-1)], 0 outside the image
        irow = ir0 + np.arange(SR) - (dy - 1)
        wcol = np.arange(W) - (dx - 1)
        rr = np.where((irow >= 0) & (irow < H))[0]
        cc = np.where((wcol >= 0) & (wcol < W))[0]
        if len(rr) and len(cc):
            body = np.zeros((SR, W), np.float32)
            body[np.ix_(rr, cc)] = mask[k][irow[rr][:, None], wcol[cc][None, :]]
            mk_host[kk, G:G + SR * W] = body.reshape(-1)
    mk_b = np.ascontiguousarray(
        np.broadcast_to(
            mk_host.reshape(1, 8 * LS).astype(ml_dtypes.float8_e4m3), (P, 8 * LS)
        )
    )
    return {
        "xin": xin_host,
        "mk": mk_b,
        "wt": wt_host,
    }


def make_in_maps(S, retina, evolve_weight, causal_mask):
    retina = np.asarray(retina, dtype=np.float32)
    Wt = np.asarray(evolve_weight, dtype=np.float32)
    mask = np.asarray(causal_mask, dtype=np.float32).reshape(9, H, W)
    wt_host = _pack_weights(Wt)
    return [_pack_core_inputs(i, S, retina, mask, wt_host) for i in range(N_CORES)]


def gather_output(results):
    outf = np.zeros((B, CB, P, H, W), np.float32)
    for core in range(N_CORES):
        band, bp = core // 2, core % 2
        o = np.asarray(results[core]["out"]).reshape(B_LOC, CB, P, OWN, W)
        outf[bp * B_LOC:(bp + 1) * B_LOC, :, :,
             band * OWN:(band + 1) * OWN, :] = o.astype(np.float32)
    return outf.reshape(B, C, H, W)


def kernel(retina, evolve_weight, causal_mask, steps):
    from concourse.bass_utils import run_bass_kernel_spmd

    S = int(steps)
    if S <= 0:
        return np.asarray(retina, dtype=np.float32).copy()
    nc = _build_program(S)
    in_maps = make_in_maps(S, retina, evolve_weight, causal_mask)
    res = run_bass_kernel_spmd(nc, in_maps, list(range(N_CORES)))
    return gather_output(res.results)


# revision 7
# speedup vs baseline: 1.3084x; 1.3084x over previous
"""Trainium2 Bass kernel for the ChimeraSurrogateNCA problem.

Masked 3x3 conv NCA, 5 steps, B=4 C=256 H=W=128, softsign residual.

Sharding: 8 cores = 4 horizontal bands (32 rows) x 2 batch pairs.
Each core keeps its 2 batches' band + steps-row halo resident in SBUF
across all steps (redundant halo compute, zero inter-core comms).
Wider bands cut the halo-redundancy overhead from 25% to 19% vs the
16-row x 4-batch split.

Layout: x stored [cin -> 2x128 partition blocks, flat (row*W + col)]
in fp16 with NO pad columns (W=128 row pitch). A 3x3 tap's dx shift is
a +-1 element offset in the flat buffer; positions that wrap into a
neighboring row are zeroed by the causal-mask tiles (host pre-shifted
per tap, 0 outside the image). Every DVE/ACT op is fully contiguous.
Masks are stored fp8e4m3 (0/1 exact) to fit the 42-row slab in SBUF.

Engine split per 4-row group (tuned against the NTFF trace):
  - mask multiplies (8 taps x 2 cin blocks) on DVE only: GPSIMD shares
    an SBUF port with the DVE, so offloading tensor_tensor work there
    slows 2-port DVE ops ~3.2x (measured) and is net-negative
  - matmuls (fp16, fp32 PSUM accum) on PE; fp8 fails accuracy (2.5-5%
    vs the 2e-2 budget, simulated)
  - softsign residual: |d| on ACT, then 1/(1+|d|) = exp(-ln(|d|+1))
    on ACT (Ln's pre-bias fuses the +1; Ln/Exp/Abs live in one ACT
    table set so there is exactly one ACT_TABLE_LOAD), d*r and the
    in-place slab add on DVE (both contiguous).
"""

import numpy as np

import concourse.bass as bass
import concourse.mybir as mybir
from concourse.tile import TileContext

F16 = mybir.dt.float16
F32 = mybir.dt.float32

N_CORES = 8
B, C, H, W = 4, 256, 128, 128
P = 128          # partitions / channel block size
CB = C // P      # channel blocks (2)
OWN = 16         # band rows owned per core
N_BANDS = H // OWN   # 8
B_LOC = 4        # batches per core; core = band index
G = 4            # guard elements on each end of a slab/mask row buffer

# taps excluding the always-unmasked center (k=4)
TAPS = [0, 1, 2, 3, 5, 6, 7, 8]


def _build_program(S, hoist=True):
    SR = OWN + 2 * S          # slab rows (42 for S=5)
    LS = G + SR * W + G       # slab flat length per partition
    nc = bass.Bass()
    xin = nc.declare_dram_parameter("xin", [B_LOC, CB, P, LS], F16, isOutput=False)
    mk = nc.declare_dram_parameter("mk", [P, 8 * LS], F16, isOutput=False)
    wt = nc.declare_dram_parameter("wt", [CB, P, 9 * CB * P], F16, isOutput=False)
    out = nc.declare_dram_parameter("out", [B_LOC, CB, P, OWN * W], F16, isOutput=True)

    with TileContext(nc) as tc:
        with (
            tc.tile_pool(name="xp", bufs=1) as xpool,
            tc.tile_pool(name="mp", bufs=1) as mpool,
            tc.tile_pool(name="wp", bufs=1) as wpool,
            tc.tile_pool(name="ap", bufs=2) as apool,
            tc.tile_pool(name="tp", bufs=3) as tpool,
            tc.tile_pool(name="pp", bufs=2, space="PSUM") as ppool,
        ):
            slab = {}
            for b in range(B_LOC):
                for cb in range(CB):
                    t = xpool.tile([P, LS], F16, tag=f"slab{b}{cb}")
                    nc.sync.dma_start(out=t[:], in_=xin[b, cb])
                    slab[b, cb] = t
            mk_sb = mpool.tile([P, 8 * LS], F16, tag="mk")
            nc.sync.dma_start(out=mk_sb[:], in_=mk[:])
            w_sb = []
            for cb in range(CB):
                t = wpool.tile([P, 9 * CB * P], F16, tag=f"w{cb}")
                nc.sync.dma_start(out=t[:], in_=wt[cb])
                w_sb.append(t)

            def w_view(k, cb, ob):
                return w_sb[cb][:, (k * CB + ob) * P:(k * CB + ob + 1) * P]

            def emit_abuild(b, r0, R):
                # a[kk,cb][i] = slab[(r0+dy-1)*W - 2 + i] * mask'[same pos]
                tiles = {}
                for kk, k in enumerate(TAPS):
                    dy = k // 3
                    o = G - 2 + (r0 + dy - 1) * W
                    n = R * W + 4
                    for cb in range(CB):
                        at = apool.tile([P, R * W + 2 * G], F16, tag=f"a{k}{cb}")
                        nc.vector.tensor_tensor(
                            out=at[:, 0:n],
                            in0=slab[b, cb][:, o:o + n],
                            in1=mk_sb[:, kk * LS + o:kk * LS + o + n],
                            op=mybir.AluOpType.mult,
                        )
                        tiles[k, cb] = at
                return tiles

            def emit_center(b, r0, R, psums):
                for ob in range(CB):
                    for cb in range(CB):
                        rhs = slab[b, cb][:, G + r0 * W:G + r0 * W + R * W]
                        nc.tensor.matmul(
                            psums[ob][:], w_view(4, cb, ob), rhs,
                            start=(cb == 0), stop=False,
                        )

            def emit_rest(b, r0, R, tiles, psums):
                for ob in range(CB):
                    n = 0
                    for k in TAPS:
                        dx = k % 3
                        for cb in range(CB):
                            n += 1
                            rhs = tiles[k, cb][:, 1 + dx:1 + dx + R * W]
                            nc.tensor.matmul(
                                psums[ob][:], w_view(k, cb, ob), rhs,
                                start=False, stop=(n == 2 * len(TAPS)),
                            )

            def emit_resid(b, r0, R, psums):
                # x += d/(1+|d|) with 1/(1+|d|) = exp(-ln(|d|+1)), all on ACT
                for ob in range(CB):
                    ps = psums[ob]
                    tabs = tpool.tile([P, R * W], F32, tag="tabs")
                    nc.scalar.activation(
                        out=tabs[:], in_=ps[:],
                        func=mybir.ActivationFunctionType.Abs,
                    )
                    v = tpool.tile([P, R * W], F32, tag="v")
                    nc.scalar.activation(
                        out=v[:], in_=tabs[:],
                        func=mybir.ActivationFunctionType.Ln, bias=1.0,
                    )
                    rt = tpool.tile([P, R * W], F32, tag="rt")
                    nc.scalar.activation(
                        out=rt[:], in_=v[:],
                        func=mybir.ActivationFunctionType.Exp, scale=-1.0,
                    )
                    st = tpool.tile([P, R * W], F16, tag="st")
                    nc.vector.tensor_tensor(
                        out=st[:], in0=ps[:], in1=rt[:], op=mybir.AluOpType.mult
                    )
                    sv = slab[b, ob][:, G + r0 * W:G + r0 * W + R * W]
                    nc.vector.tensor_tensor(
                        out=sv, in0=sv, in1=st[:], op=mybir.AluOpType.add
                    )

            for t in range(1, S + 1):
                lo, hi = t, SR - t
                for b in range(B_LOC):
                    groups = []
                    r = lo
                    while r < hi:
                        Rg = min(4, hi - r)
                        groups.append((r, Rg))
                        r += Rg
                    pending = None
                    for (r0, Rg) in groups:
                        tiles = emit_abuild(b, r0, Rg)
                        psums = [
                            ppool.tile([P, Rg * W], F32, tag=f"ps{ob}", name=f"ps{ob}")
                            for ob in range(CB)
                        ]
                        emit_center(b, r0, Rg, psums)
                        if pending is not None:
                            emit_resid(b, *pending)
                        emit_rest(b, r0, Rg, tiles, psums)
                        pending = (r0, Rg, psums)
                    emit_resid(b, *pending)

            for b in range(B_LOC):
                for cb in range(CB):
                    nc.sync.dma_start(
                        out=out[b, cb],
                        in_=slab[b, cb][:, G + S * W:G + (S + OWN) * W],
                    )
    if hoist:
        _hoist_extra_waits(nc)
    return nc


# Engine compute instructions have a single hardware sync-wait slot on
# trn2 (walrus: "Too many sync wait commands"); Tile may attach 2-3.
# Hoist the extras onto standalone EventSemaphore waits on the same
# engine queue immediately before the instruction.
_NO_HOIST = {
    "InstEventSemaphore", "InstCall",
    "InstUnconditionalBranch", "InstRegisterMove",
}


def _hoist_extra_waits(nc, max_waits=1):
    fn = nc.m.functions[0]
    n = 0
    for blk in fn.blocks:
        newlist = []
        for inst in blk.instructions:
            if (
                type(inst).__name__ == "InstISA"
                and getattr(inst, "op_name", "") == "EVENT_SEMAPHORE_RANGE_CLEAR"
            ):
                # kernel-tail lazy-sem reset; this walrus can't encode
                # opcode 176 ("ISA wrong length"). Only needed for NEFF
                # re-execution, which the runtime handles via fresh loads.
                continue
            si = inst.sync_info
            if (
                si is not None
                and si.on_wait
                and len(si.on_wait) > max_waits
                and type(inst).__name__ not in _NO_HOIST
            ):
                waits = list(si.on_wait)
                extra, keep = waits[:-max_waits], waits[-max_waits:]
                for j, wsub in enumerate(extra):
                    carrier = mybir.InstEventSemaphore(
                        name=f"hwait-{inst.name}-{j}", ins=[], outs=[]
                    )
                    carrier.engine = inst.engine
                    carrier.sync_info = type(si)(on_wait=[wsub], on_update=[])
                    newlist.append(carrier)
                    n += 1
                inst.sync_info = type(si)(
                    on_wait=keep, on_update=list(si.on_update or [])
                )
            newlist.append(inst)
        try:
            blk.instructions = newlist
        except Exception:
            blk.instructions[:] = newlist
    return n


def _pack_weights(Wt):
    # wt[cb][p, k*2*P + ob*P + co] = Wt[ob*P + co, cb*P + p, k]
    Wr = np.ascontiguousarray(Wt.reshape(C, C, 9))
    wta = Wr.reshape(CB, P, CB, P, 9)            # [ob, co, cb, p, k]
    wta = wta.transpose(2, 3, 4, 0, 1)           # [cb, p, k, ob, co]
    return np.ascontiguousarray(wta.reshape(CB, P, 9 * CB * P)).astype(np.float16)


def _pack_core_inputs(core, S, retina, mask, wt_host):
    SR = OWN + 2 * S
    LS = G + SR * W + G
    band = core
    bsel = slice(0, B)
    ir0 = band * OWN - S  # image row of slab row 0
    xin_host = np.zeros((B_LOC, CB, P, LS), np.float16)
    rlo = max(0, -ir0)
    rhi = min(SR, H - ir0)
    if rhi > rlo:
        xin_host[:, :, :, G + rlo * W:G + rhi * W] = (
            retina.reshape(B, CB, P, H, W)[bsel, :, :, ir0 + rlo:ir0 + rhi, :]
            .astype(np.float16).reshape(B_LOC, CB, P, (rhi - rlo) * W)
        )
    mk_host = np.zeros((8, LS), np.float32)
    for kk, k in enumerate(TAPS):
        dy, dx = k // 3, k % 3
        # mask multiplying slab input element (q, c) for tap k is
        # mask[k][ir0 + q - (dy-1), c - (dx-1)], 0 outside the image
        irow = ir0 + np.arange(SR) - (dy - 1)
        wcol = np.arange(W) - (dx - 1)
        rr = np.where((irow >= 0) & (irow < H))[0]
        cc = np.where((wcol >= 0) & (wcol < W))[0]
        if len(rr) and len(cc):
            body = np.zeros((SR, W), np.float32)
            body[np.ix_(rr, cc)] = mask[k][irow[rr][:, None], wcol[cc][None, :]]
            mk_host[kk, G:G + SR * W] = body.reshape(-1)
    mk_b = np.ascontiguousarray(
        np.broadcast_to(
            mk_host.reshape(1, 8 * LS).astype(np.float16), (P, 8 * LS)
        )
    )
    return {
        "xin": xin_host,
        "mk": mk_b,
        "wt": wt_host,
    }


def make_in_maps(S, retina, evolve_weight, causal_mask):
    retina = np.asarray(retina, dtype=np.float32)
    Wt = np.asarray(evolve_weight, dtype=np.float32)
    mask = np.asarray(causal_mask, dtype=np.float32).reshape(9, H, W)
    wt_host = _pack_weights(Wt)
    return [_pack_core_inputs(i, S, retina, mask, wt_host) for i in range(N_CORES)]


def gather_output(results):
    outf = np.zeros((B, CB, P, H, W), np.float32)
    for core in range(N_CORES):
        band = core
        o = np.asarray(results[core]["out"]).reshape(B_LOC, CB, P, OWN, W)
        outf[:, :, :, band * OWN:(band + 1) * OWN, :] = o.astype(np.float32)
    return outf.reshape(B, C, H, W)


def kernel(retina, evolve_weight, causal_mask, steps):
    from concourse.bass_utils import run_bass_kernel_spmd

    S = int(steps)
    if S <= 0:
        return np.asarray(retina, dtype=np.float32).copy()
    nc = _build_program(S)
    in_maps = make_in_maps(S, retina, evolve_weight, causal_mask)
    res = run_bass_kernel_spmd(nc, in_maps, list(range(N_CORES)))
    return gather_output(res.results)
